# revision 5
# baseline (speedup 1.0000x reference)
"""Trainium2 Bass kernel for per-node masked MLP (gnn_message_passing).

Reference computation (B=8192 batch, T=128 nodes, H=64 hidden, C=2 out):
    h   = leaky_relu(einsum('tij,jt,bj->bti', w0, adj, x) + b0)   adj = 1-eye
    h   = leaky_relu(einsum('tij,btj->bti', w1, h) + b1)
    out = einsum('tij,btj->bti', w2, h) + b2

Strategy: data-parallel over batch across 8 NeuronCores (1024 rows each).
Per core, all three layers are expressed as TensorE matmuls with the
(t,i)/(t-pair) axes laid out on PSUM partitions and batch streaming on the
moving free dim (fp32r inputs -> full-rate PE):
  L0: one [j=128, ti-tile=128] stationary per 128-wide ti block (the
      self-loop mask is folded into the weights host-side).
  L1: block-diagonal [W1[2m].T (+) W1[2m+1].T] stationary per node pair.
  L2: 128-wide stationary accumulating 32 node pairs into one PSUM bank
      (each pair owns a distinct 4-column strip).
Bias + leaky-relu ride the mandatory PSUM->SBUF evacuation (ScalarE Lrelu,
with a share of tiles on VectorE as tensor_scalar/scalar_tensor_tensor to
balance the two engines).
"""

import sys

if "/opt/trn_rl_repo" not in sys.path:
    sys.path.insert(0, "/opt/trn_rl_repo")

import numpy as np

B = 8192
T = 128
H = 64
C = 2
N_CORES = 8
BC = B // N_CORES  # 1024 batch rows per core
M_TILES = 64  # 128-wide (t,i) tiles for L0 == node pairs for L1/L2
NEG = 0.01  # leaky_relu negative slope


def _split_sync_waits(nc, cap=1):
    """This container's walrus build encodes at most ~1 sync wait per
    instruction (setupSyncWait: "Too many sync wait commands"), while Tile's
    sem assignment freely attaches several. Post-pass: leave `cap` waits on
    each instruction and hoist the extras onto single-wait NOPs inserted
    just before it on the same engine (same-engine FIFO preserves
    semantics)."""
    from concourse import mybir

    ctr = [0]
    for f in nc.m.functions:
        for blk in f.blocks:
            new_list = []
            for ins in blk.instructions:
                si = getattr(ins, "sync_info", None)
                waits = list(si.on_wait) if si is not None and si.on_wait else []
                if len(waits) > cap:
                    keep = waits[:cap]
                    extra = waits[cap:]
                    for w in extra:
                        ctr[0] += 1
                        nop = mybir.InstNoOp(
                            name=f"{ins.name}-ws{ctr[0]}",
                            engine=ins.engine,
                            ins=[],
                            outs=[],
                            sync_info=mybir.SyncInfo(on_wait=[w], on_update=[]),
                        )
                        new_list.append(nop)
                    ins.sync_info = mybir.SyncInfo(
                        on_wait=keep, on_update=list(si.on_update or [])
                    )
                new_list.append(ins)
            blk.instructions[:] = new_list


def build_program(loop_R=None):
    """Build the per-core Bass program. loop_R wraps the whole body in a
    hardware For_i loop (used only for wall-clock slope timing)."""
    import concourse.bass as bass
    import concourse.tile as tile
    from concourse import mybir

    f32 = mybir.dt.float32
    f32r = mybir.dt.float32r
    Alu = mybir.AluOpType
    Act = mybir.ActivationFunctionType

    nc = bass.Bass()
    xt_d = nc.dram_tensor("xt", [T, BC], f32r, kind="ExternalInput")
    w0_d = nc.dram_tensor("w0s", [M_TILES, T, T], f32r, kind="ExternalInput")
    w1_d = nc.dram_tensor("w1s", [M_TILES, T, T], f32r, kind="ExternalInput")
    w2_d = nc.dram_tensor("w2s", [M_TILES, T, T], f32r, kind="ExternalInput")
    b0_d = nc.dram_tensor("b0s", [T, M_TILES], f32, kind="ExternalInput")
    b1_d = nc.dram_tensor("b1s", [T, M_TILES], f32, kind="ExternalInput")
    b1p_d = nc.dram_tensor("b1p", [T, M_TILES], f32, kind="ExternalInput")
    b2_d = nc.dram_tensor("b2s", [T, 2], f32, kind="ExternalInput")
    out_d = nc.dram_tensor("out", [2 * T, BC], f32, kind="ExternalOutput")

    with tile.TileContext(nc) as tc:
        with (
            tc.tile_pool(name="const", bufs=1) as cp,
            tc.tile_pool(name="w0p", bufs=2) as w0p,
            tc.tile_pool(name="w1p", bufs=2) as w1p,
            tc.tile_pool(name="w2p", bufs=2) as w2p,
            tc.tile_pool(name="h0p", bufs=2) as h0p,
            tc.tile_pool(name="h1p", bufs=2) as h1p,
            tc.tile_pool(name="tmp", bufs=3) as tmpp,
            tc.tile_pool(name="outp", bufs=2) as outp,
            tc.tile_pool(name="psA", bufs=3, space=bass.MemorySpace.PSUM) as psA,
            tc.tile_pool(name="psB", bufs=3, space=bass.MemorySpace.PSUM) as psB,
            tc.tile_pool(name="psCa", bufs=1, space=bass.MemorySpace.PSUM) as psCa,
            tc.tile_pool(name="psCb", bufs=1, space=bass.MemorySpace.PSUM) as psCb,
        ):
            xtt = cp.tile([T, BC], f32r, tag="xt")
            nc.sync.dma_start(xtt[:], xt_d[:])
            b0t = cp.tile([T, M_TILES], f32, tag="b0")
            nc.sync.dma_start(b0t[:], b0_d[:])
            b1t = cp.tile([T, M_TILES], f32, tag="b1")
            nc.sync.dma_start(b1t[:], b1_d[:])
            b1pt = cp.tile([T, M_TILES], f32, tag="b1p")
            nc.sync.dma_start(b1pt[:], b1p_d[:])
            b2t = cp.tile([T, 2], f32, tag="b2")
            nc.sync.dma_start(b2t[:], b2_d[:])

            # VectorE handles this fraction of the leaky evacuations so the
            # two evacuation engines finish together
            # (ACT ~570ns vs DVE 2-op ~1316ns per [128,512] tile).
            dve_ctr = [0]

            def evac_leaky(dst, ps, bias_col, bias01_col):
                k = dve_ctr[0]
                dve_ctr[0] += 1
                if k % 13 < 8 and bias01_col is not None:
                    t1 = tmpp.tile([T, 512], f32, tag="t1")
                    nc.vector.tensor_scalar(
                        t1[:], ps, NEG, bias01_col, op0=Alu.mult, op1=Alu.add
                    )
                    nc.vector.scalar_tensor_tensor(
                        dst, ps, bias_col, t1[:], op0=Alu.add, op1=Alu.max
                    )
                else:
                    nc.scalar.activation(
                        dst, ps, Act.Lrelu, bias=bias_col, scale=1.0, alpha=NEG
                    )

            def body(_iv=None):
                for m in range(M_TILES):
                    g, mq = divmod(m, 32)
                    w0t = w0p.tile([T, T], f32r, tag="w0")
                    nc.sync.dma_start(w0t[:], w0_d[m])
                    w1t = w1p.tile([T, T], f32r, tag="w1")
                    nc.sync.dma_start(w1t[:], w1_d[m])
                    w2t = w2p.tile([T, T], f32r, tag="w2")
                    nc.sync.dma_start(w2t[:], w2_d[m])

                    if mq == 0:
                        ps2a = psCa.tile([T, 512], f32, tag="ps2a")
                        ps2b = psCb.tile([T, 512], f32, tag="ps2b")
                        body.ps2 = (ps2a, ps2b)
                    ps2a, ps2b = body.ps2

                    # ---- layer 0 ----
                    ps0a = psA.tile([T, 512], f32, tag="ps0")
                    nc.tensor.matmul(
                        ps0a[:], w0t[:], xtt[:, 0:512],
                        start=True, stop=True,
                    )
                    ps0b = psA.tile([T, 512], f32, tag="ps0")
                    nc.tensor.matmul(
                        ps0b[:], w0t[:], xtt[:, 512:1024],
                        start=True, stop=True,
                    )
                    h0 = h0p.tile([T, BC], f32r, tag="h0")
                    nc.scalar.activation(
                        h0[:, 0:512], ps0a[:], Act.Lrelu,
                        bias=b0t[:, m : m + 1], scale=1.0, alpha=NEG,
                    )
                    nc.scalar.activation(
                        h0[:, 512:1024], ps0b[:], Act.Lrelu,
                        bias=b0t[:, m : m + 1], scale=1.0, alpha=NEG,
                    )

                    # ---- layer 1 ----
                    ps1a = psB.tile([T, 512], f32, tag="ps1")
                    nc.tensor.matmul(
                        ps1a[:], w1t[:], h0[:, 0:512],
                        start=True, stop=True,
                    )
                    ps1b = psB.tile([T, 512], f32, tag="ps1")
                    nc.tensor.matmul(
                        ps1b[:], w1t[:], h0[:, 512:1024],
                        start=True, stop=True,
                    )
                    h1 = h1p.tile([T, BC], f32r, tag="h1")
                    evac_leaky(h1[:, 0:512], ps1a[:], b1t[:, m : m + 1], b1pt[:, m : m + 1])
                    evac_leaky(h1[:, 512:1024], ps1b[:], b1t[:, m : m + 1], b1pt[:, m : m + 1])

                    # ---- layer 2 (accumulate 32 pairs into one bank) ----
                    nc.tensor.matmul(
                        ps2a[:], w2t[:], h1[:, 0:512],
                        start=(mq == 0), stop=(mq == 31),
                    )
                    nc.tensor.matmul(
                        ps2b[:], w2t[:], h1[:, 512:1024],
                        start=(mq == 0), stop=(mq == 31),
                    )

                    if mq == 31:
                        oa = outp.tile([T, 512], f32, tag="oa")
                        nc.scalar.activation(
                            oa[:], ps2a[:], Act.Identity, bias=b2t[:, g : g + 1]
                        )
                        nc.sync.dma_start(out_d[128 * g : 128 * (g + 1), 0:512], oa[:])
                        ob = outp.tile([T, 512], f32, tag="ob")
                        nc.scalar.activation(
                            ob[:], ps2b[:], Act.Identity, bias=b2t[:, g : g + 1]
                        )
                        nc.sync.dma_start(
                            out_d[128 * g : 128 * (g + 1), 512:1024], ob[:]
                        )

            if loop_R is None:
                body()
            else:
                with tc.For_i(0, loop_R, 1) as iv:
                    body(iv)

    _split_sync_waits(nc)
    return nc


def prep_inputs(x, w0, b0, w1, b1, w2, b2):
    """Host-side reshuffle of the full inputs into the per-core tensors."""
    x = np.ascontiguousarray(np.asarray(x, dtype=np.float32))
    w0 = np.asarray(w0, dtype=np.float32)
    b0 = np.asarray(b0, dtype=np.float32)
    w1 = np.asarray(w1, dtype=np.float32)
    b1 = np.asarray(b1, dtype=np.float32)
    w2 = np.asarray(w2, dtype=np.float32)
    b2 = np.asarray(b2, dtype=np.float32)

    # L0 stationaries: mask self-loop, lay out as [m, j, p] with p -> ti=128m+p
    w0m = w0.copy()
    w0m[np.arange(T), :, np.arange(T)] = 0.0
    w0all = w0m.transpose(2, 0, 1).reshape(T, T * H)  # [j, ti]
    w0s = np.ascontiguousarray(
        w0all.reshape(T, M_TILES, T).transpose(1, 0, 2)
    )  # [m, j, p]

    # L1 stationaries: block-diag of the pair's transposed weights
    w1T = w1.transpose(0, 2, 1)  # [t, i_in, i_out]
    w1s = np.zeros((M_TILES, T, T), np.float32)
    w1s[:, :H, :H] = w1T[0::2]
    w1s[:, H:, H:] = w1T[1::2]

    # L2 stationaries: pair m owns columns 4*(m%32) .. +4
    w2T = w2.transpose(0, 2, 1)  # [t, i, c]
    w2s = np.zeros((M_TILES, T, T), np.float32)
    for m in range(M_TILES):
        col = 4 * (m % 32)
        w2s[m, :H, col : col + C] = w2T[2 * m]
        w2s[m, H:, col + C : col + 2 * C] = w2T[2 * m + 1]

    b0s = np.ascontiguousarray(b0.reshape(-1).reshape(M_TILES, T).T)
    b1s = np.ascontiguousarray(b1.reshape(-1).reshape(M_TILES, T).T)
    b1p = np.ascontiguousarray(NEG * b1s)
    b2s = np.ascontiguousarray(b2.reshape(-1).reshape(2, T).T)

    shared = {
        "w0s": w0s, "w1s": w1s, "w2s": w2s,
        "b0s": b0s, "b1s": b1s, "b1p": b1p, "b2s": b2s,
    }
    in_maps = []
    for c in range(N_CORES):
        xt_c = np.ascontiguousarray(x[c * BC : (c + 1) * BC].T)  # [128, BC]
        in_maps.append({"xt": xt_c, **shared})
    return in_maps


def gather_output(results):
    """results: list of per-core {"out": [256, BC]} -> full [B, T, C]."""
    parts = []
    for c in range(N_CORES):
        o = np.asarray(results[c]["out"])  # [2T, BC], row r = t*2+c
        parts.append(o.reshape(T, C, BC).transpose(2, 0, 1))
    return np.ascontiguousarray(np.concatenate(parts, axis=0))


_NC_CACHE = {}


def kernel(x, w0, b0, w1, b1, w2, b2):
    from concourse.bass_utils import run_bass_kernel_spmd

    if "nc" not in _NC_CACHE:
        _NC_CACHE["nc"] = build_program()
    nc = _NC_CACHE["nc"]
    in_maps = prep_inputs(x, w0, b0, w1, b1, w2, b2)
    res = run_bass_kernel_spmd(nc, in_maps, core_ids=list(range(N_CORES)))
    return gather_output(res.results)


# revision 15
# speedup vs baseline: 7.6840x; 7.6840x over previous
"""Trainium2 Bass kernel for per-node masked MLP (gnn_message_passing).

Reference computation (B=8192 batch, T=128 nodes, H=64 hidden, C=2 out):
    h   = leaky_relu(einsum('tij,jt,bj->bti', w0, adj, x) + b0)   adj = 1-eye
    h   = leaky_relu(einsum('tij,btj->bti', w1, h) + b1)
    out = einsum('tij,btj->bti', w2, h) + b2

Strategy: data-parallel over batch across 8 NeuronCores (1024 rows each).
Per core, all three layers are TensorE matmuls with the (t,i) axes on PSUM
partitions and batch streaming on the moving free dim (fp32r -> full-rate
PE). All weights are preloaded into SBUF once (12 MB resident).
  L0: [j=128, ti-tile=128] stationary per 128-wide ti block (self-loop mask
      folded into the weights host-side).
  L1: block-diagonal [W1[2m].T (+) W1[2m+1].T] stationary per node pair.
  L2: 128-wide stationary accumulating 32 node pairs into one PSUM bank
      (each pair owns a distinct 4-column strip).
Bias + leaky-relu ride the PSUM->SBUF evacuation, balanced across ScalarE
(Lrelu) and VectorE (fused custom DVE op, or a 2-op fallback).
"""

import sys

if "/opt/trn_rl_repo" not in sys.path:
    sys.path.insert(0, "/opt/trn_rl_repo")

import numpy as np

B = 8192
T = 128
H = 64
C = 2
N_CORES = 8
BC = B // N_CORES  # 1024 batch rows per core
M_TILES = 64  # 128-wide (t,i) tiles for L0 == node pairs for L1/L2
NEG = 0.01  # leaky_relu negative slope

LEAKY_OP_NAME = "LEAKY_BIAS_ANT"


def _register_leaky_op():
    """Custom fused DVE op: out = max(in0 + s0, in0*imm2 + s1).
    With s0=bias, s1=NEG*bias, imm2=NEG this is leaky_relu(in0+bias) in one
    VectorE pass. Registered into concourse.dve_ops.OPS at runtime."""
    import concourse.dve_ops as dve_ops
    from concourse.dve_ops import DveOp
    from concourse.dve_spec import C0, C1, C2, Spec, Src0, lower, maxx
    from concourse.dve_uop import DveOpSpec

    for op in dve_ops.OPS:
        if op.name == LEAKY_OP_NAME:
            return op

    def _reference(in0, in1, s0, s1, imm2):
        z = in0.astype(np.float32)
        return np.maximum(z + s0, z * imm2 + s1).astype(np.float32)

    spec = Spec(body=maxx(Src0 + C0, Src0 * C2 + C1), reference=_reference)
    row = dve_ops._CUSTOM_DVE_ROW_BASE + len(dve_ops.OPS)
    assert row < 0x20
    shas = {}
    for ver in ("v3", "v4"):
        s = DveOpSpec(
            name=LEAKY_OP_NAME, opcode=row, uops=lower(spec, ver=ver), rd1_en=False
        )
        shas[ver] = s.sha(ver)
    op = DveOp(LEAKY_OP_NAME, spec, subdim=False, uops_sha=shas)
    dve_ops.OPS.append(op)
    dve_ops._SUB_OPCODE_FOR_NAME[LEAKY_OP_NAME] = row
    dve_ops.CUSTOM_DVE_SPECS[LEAKY_OP_NAME] = spec
    return op


def _split_sync_waits(nc, cap=1):
    """This container's walrus build encodes at most ~1 sync wait per
    instruction (setupSyncWait: "Too many sync wait commands"), while Tile's
    sem assignment freely attaches several. Post-pass: leave `cap` waits on
    each instruction and hoist the extras onto single-wait NOPs inserted
    just before it on the same engine (same-engine FIFO preserves
    semantics)."""
    from concourse import mybir

    ctr = [0]
    for f in nc.m.functions:
        for blk in f.blocks:
            new_list = []
            for ins in blk.instructions:
                si = getattr(ins, "sync_info", None)
                waits = list(si.on_wait) if si is not None and si.on_wait else []
                if len(waits) > cap:
                    keep = waits[:cap]
                    extra = waits[cap:]
                    for w in extra:
                        ctr[0] += 1
                        nop = mybir.InstNoOp(
                            name=f"{ins.name}-ws{ctr[0]}",
                            engine=ins.engine,
                            ins=[],
                            outs=[],
                            sync_info=mybir.SyncInfo(on_wait=[w], on_update=[]),
                        )
                        new_list.append(nop)
                    ins.sync_info = mybir.SyncInfo(
                        on_wait=keep, on_update=list(si.on_update or [])
                    )
                new_list.append(ins)
            blk.instructions[:] = new_list


def build_program(loop_R=None, evac="mixed", l2="wide", dve_frac=None, evac_width=512):
    """Build the per-core Bass program.

    loop_R: wrap the body in a hardware For_i loop (wall-clock slope timing).
    evac:  "mixed" (ACT Lrelu + custom DVE op, balanced), "act", "dvec"
           (custom DVE only), "dve" (2-op DVE only), "none" (timing probe:
           matmuls run on stale h tiles, no evacuation work).
    l2:    "wide" | "off" (timing probe).
    """
    import concourse.bass as bass
    import concourse.tile as tile
    from concourse import mybir

    f32 = mybir.dt.float32
    f32r = mybir.dt.float32r
    Alu = mybir.AluOpType
    Act = mybir.ActivationFunctionType

    nc = bass.Bass()
    xt_d = nc.dram_tensor("xt", [T, BC], f32r, kind="ExternalInput")
    w0_d = nc.dram_tensor("w0w", [T, M_TILES * T], f32r, kind="ExternalInput")
    w1_d = nc.dram_tensor("w1w", [T, M_TILES * T], f32r, kind="ExternalInput")
    w2_d = nc.dram_tensor("w2w", [T, M_TILES * T], f32r, kind="ExternalInput")
    b0_d = nc.dram_tensor("b0s", [T, M_TILES], f32, kind="ExternalInput")
    b1_d = nc.dram_tensor("b1s", [T, M_TILES], f32, kind="ExternalInput")
    b1p_d = nc.dram_tensor("b1p", [T, M_TILES], f32, kind="ExternalInput")
    b0p_d = nc.dram_tensor("b0p", [T, M_TILES], f32, kind="ExternalInput")
    b2_d = nc.dram_tensor("b2s", [T, 2], f32, kind="ExternalInput")
    out_d = nc.dram_tensor("out", [2 * T, BC], f32, kind="ExternalOutput")

    with tile.TileContext(nc) as tc:
        with (
            tc.tile_pool(name="const", bufs=1) as cp,
            tc.tile_pool(name="h0p", bufs=2) as h0p,
            tc.tile_pool(name="h1p", bufs=2) as h1p,
            tc.tile_pool(name="tmp", bufs=3) as tmpp,
            tc.tile_pool(name="outp", bufs=2) as outp,
            tc.tile_pool(name="psA", bufs=3, space=bass.MemorySpace.PSUM) as psA,
            tc.tile_pool(name="psB", bufs=3, space=bass.MemorySpace.PSUM) as psB,
            tc.tile_pool(name="psCa", bufs=1, space=bass.MemorySpace.PSUM) as psCa,
            tc.tile_pool(name="psCb", bufs=1, space=bass.MemorySpace.PSUM) as psCb,
        ):
            # ---- resident tensors (loaded once) ----
            xtt = cp.tile([T, BC], f32r, tag="xt")
            nc.sync.dma_start(xtt[:], xt_d[:])
            w0sb = cp.tile([T, M_TILES * T], f32r, tag="w0w")
            nc.sync.dma_start(w0sb[:], w0_d[:])
            w1sb = cp.tile([T, M_TILES * T], f32r, tag="w1w")
            nc.sync.dma_start(w1sb[:], w1_d[:])
            w2sb = cp.tile([T, M_TILES * T], f32r, tag="w2w")
            nc.sync.dma_start(w2sb[:], w2_d[:])
            b0t = cp.tile([T, M_TILES], f32, tag="b0")
            nc.sync.dma_start(b0t[:], b0_d[:])
            b0pt = cp.tile([T, M_TILES], f32, tag="b0p")
            nc.sync.dma_start(b0pt[:], b0p_d[:])
            b1t = cp.tile([T, M_TILES], f32, tag="b1")
            nc.sync.dma_start(b1t[:], b1_d[:])
            b1pt = cp.tile([T, M_TILES], f32, tag="b1p")
            nc.sync.dma_start(b1pt[:], b1p_d[:])
            b2t = cp.tile([T, 2], f32, tag="b2")
            nc.sync.dma_start(b2t[:], b2_d[:])

            if evac == "none":
                h0fix = cp.tile([T, BC], f32r, tag="h0fix")
                nc.gpsimd.memset(h0fix[:].bitcast(f32), 0.125)
                h1fix = cp.tile([T, BC], f32r, tag="h1fix")
                nc.gpsimd.memset(h1fix[:].bitcast(f32), 0.125)
            if evac == "empty":
                scratch = cp.tile([T, 16], f32, tag="scratch")

            # ACT Lrelu ~570ns vs DVE 2-op ~1316ns per [128,512] tile:
            # give VectorE ~31% of the evacuations (4/13).
            # (A fused custom DVE op would halve the DVE cost, but this
            # walrus build rejects InstCustomDveAnt: "ISA wrong length".)
            if dve_frac is None:
                n_dve, n_mod = (4, 13) if evac_width == 512 else (3, 10)
            else:
                n_dve, n_mod = dve_frac
            ctr = [0]

            def evac_leaky(dst, ps, bias_col, bias01_col):
                k = ctr[0]
                ctr[0] += 1
                use_dve = (evac in ("dvec", "dve")) or (
                    evac == "mixed" and k % n_mod < n_dve
                )
                if use_dve:
                    t1 = tmpp.tile([T, evac_width], f32, tag="t1")
                    nc.vector.tensor_scalar(
                        t1[:], ps, NEG, bias01_col, op0=Alu.mult, op1=Alu.add
                    )
                    nc.vector.scalar_tensor_tensor(
                        dst, ps, bias_col, t1[:], op0=Alu.add, op1=Alu.max
                    )
                else:
                    nc.scalar.activation(
                        dst, ps, Act.Lrelu, bias=bias_col, scale=1.0, alpha=NEG
                    )

            def body(_iv=None):
                if evac == "empty":
                    nc.gpsimd.memset(scratch[:], 0.0)
                    return
                for m in range(M_TILES):
                    g, mq = divmod(m, 32)
                    w0t = w0sb[:, T * m : T * (m + 1)]
                    w1t = w1sb[:, T * m : T * (m + 1)]
                    w2t = w2sb[:, T * m : T * (m + 1)]

                    if l2 == "wide" and mq == 0:
                        ps2a = psCa.tile([T, 512], f32, tag="ps2a")
                        ps2b = psCb.tile([T, 512], f32, tag="ps2b")
                        body.ps2 = (ps2a, ps2b)

                    # ---- layer 0 ----
                    if evac_width == 1024:
                        ps0 = psA.tile([T, 1024], f32, tag="ps")
                        nc.tensor.matmul(
                            ps0[:, 0:512], w0t, xtt[:, 0:512], start=True, stop=True
                        )
                        nc.tensor.matmul(
                            ps0[:, 512:1024], w0t, xtt[:, 512:1024],
                            start=True, stop=True,
                        )
                        ps0_parts = [(ps0[:, 0:1024], slice(0, 1024))]
                    else:
                        ps0a = psA.tile([T, 512], f32, tag="ps0")
                        nc.tensor.matmul(
                            ps0a[:], w0t, xtt[:, 0:512], start=True, stop=True
                        )
                        ps0b = psA.tile([T, 512], f32, tag="ps0")
                        nc.tensor.matmul(
                            ps0b[:], w0t, xtt[:, 512:1024], start=True, stop=True
                        )
                        ps0_parts = [(ps0a[:], slice(0, 512)), (ps0b[:], slice(512, 1024))]
                    if evac == "none":
                        h0 = h0fix
                    else:
                        h0 = h0p.tile([T, BC], f32r, tag="h0")
                        for ps_ap, sl in ps0_parts:
                            evac_leaky(h0[:, sl], ps_ap, b0t[:, m : m + 1], b0pt[:, m : m + 1])

                    # ---- layer 1 ----
                    if evac_width == 1024:
                        ps1 = psA.tile([T, 1024], f32, tag="ps")
                        nc.tensor.matmul(
                            ps1[:, 0:512], w1t, h0[:, 0:512], start=True, stop=True
                        )
                        nc.tensor.matmul(
                            ps1[:, 512:1024], w1t, h0[:, 512:1024],
                            start=True, stop=True,
                        )
                        ps1_parts = [(ps1[:, 0:1024], slice(0, 1024))]
                    else:
                        ps1a = psB.tile([T, 512], f32, tag="ps1")
                        nc.tensor.matmul(
                            ps1a[:], w1t, h0[:, 0:512], start=True, stop=True
                        )
                        ps1b = psB.tile([T, 512], f32, tag="ps1")
                        nc.tensor.matmul(
                            ps1b[:], w1t, h0[:, 512:1024], start=True, stop=True
                        )
                        ps1_parts = [(ps1a[:], slice(0, 512)), (ps1b[:], slice(512, 1024))]
                    if evac == "none":
                        h1 = h1fix
                    else:
                        h1 = h1p.tile([T, BC], f32r, tag="h1")
                        for ps_ap, sl in ps1_parts:
                            evac_leaky(h1[:, sl], ps_ap, b1t[:, m : m + 1], b1pt[:, m : m + 1])

                    # ---- layer 2 (accumulate 32 pairs into one bank) ----
                    if l2 == "wide":
                        ps2a, ps2b = body.ps2
                        nc.tensor.matmul(
                            ps2a[:], w2t, h1[:, 0:512],
                            start=(mq == 0), stop=(mq == 31),
                        )
                        nc.tensor.matmul(
                            ps2b[:], w2t, h1[:, 512:1024],
                            start=(mq == 0), stop=(mq == 31),
                        )
                        if mq == 31:
                            oa = outp.tile([T, 512], f32, tag="oa")
                            nc.scalar.activation(
                                oa[:], ps2a[:], Act.Identity, bias=b2t[:, g : g + 1]
                            )
                            nc.sync.dma_start(
                                out_d[128 * g : 128 * (g + 1), 0:512], oa[:]
                            )
                            ob = outp.tile([T, 512], f32, tag="ob")
                            nc.scalar.activation(
                                ob[:], ps2b[:], Act.Identity, bias=b2t[:, g : g + 1]
                            )
                            nc.sync.dma_start(
                                out_d[128 * g : 128 * (g + 1), 512:1024], ob[:]
                            )

            if loop_R is None:
                body()
            else:
                with tc.For_i(0, loop_R, 1) as iv:
                    body(iv)

            if evac == "empty" or l2 == "off":
                # timing probes never write out_d in the body; bind it so the
                # output tensor isn't dead
                z = cp.tile([T, 16], f32, tag="zpad")
                nc.gpsimd.memset(z[:], 0.0)
                nc.sync.dma_start(out_d[0:T, 0:16], z[:])

    _split_sync_waits(nc)
    return nc


def prep_inputs(x, w0, b0, w1, b1, w2, b2):
    """Host-side reshuffle of the full inputs into the per-core tensors."""
    x = np.ascontiguousarray(np.asarray(x, dtype=np.float32))
    w0 = np.asarray(w0, dtype=np.float32)
    b0 = np.asarray(b0, dtype=np.float32)
    w1 = np.asarray(w1, dtype=np.float32)
    b1 = np.asarray(b1, dtype=np.float32)
    w2 = np.asarray(w2, dtype=np.float32)
    b2 = np.asarray(b2, dtype=np.float32)

    # L0 stationaries: mask self-loop; [j, (m p)] with column 128m+p -> ti
    w0m = w0.copy()
    w0m[np.arange(T), :, np.arange(T)] = 0.0
    w0w = np.ascontiguousarray(w0m.transpose(2, 0, 1).reshape(T, T * H))

    # L1 stationaries: block-diag of the pair's transposed weights
    w1T = w1.transpose(0, 2, 1)  # [t, i_in, i_out]
    w1s = np.zeros((M_TILES, T, T), np.float32)
    w1s[:, :H, :H] = w1T[0::2]
    w1s[:, H:, H:] = w1T[1::2]
    w1w = np.ascontiguousarray(w1s.transpose(1, 0, 2).reshape(T, M_TILES * T))

    # L2 stationaries: pair m owns columns 4*(m%32) .. +4
    w2T = w2.transpose(0, 2, 1)  # [t, i, c]
    w2s = np.zeros((M_TILES, T, T), np.float32)
    for m in range(M_TILES):
        col = 4 * (m % 32)
        w2s[m, :H, col : col + C] = w2T[2 * m]
        w2s[m, H:, col + C : col + 2 * C] = w2T[2 * m + 1]
    w2w = np.ascontiguousarray(w2s.transpose(1, 0, 2).reshape(T, M_TILES * T))

    b0s = np.ascontiguousarray(b0.reshape(-1).reshape(M_TILES, T).T)
    b1s = np.ascontiguousarray(b1.reshape(-1).reshape(M_TILES, T).T)
    b2s = np.ascontiguousarray(b2.reshape(-1).reshape(2, T).T)

    shared = {
        "w0w": w0w, "w1w": w1w, "w2w": w2w,
        "b0s": b0s, "b0p": np.ascontiguousarray(NEG * b0s),
        "b1s": b1s, "b1p": np.ascontiguousarray(NEG * b1s),
        "b2s": b2s,
    }
    in_maps = []
    for c in range(N_CORES):
        xt_c = np.ascontiguousarray(x[c * BC : (c + 1) * BC].T)  # [128, BC]
        in_maps.append({"xt": xt_c, **shared})
    return in_maps


def gather_output(results):
    """results: list of per-core {"out": [256, BC]} -> full [B, T, C]."""
    parts = []
    for c in range(N_CORES):
        o = np.asarray(results[c]["out"])  # [2T, BC], row r = t*2+c
        parts.append(o.reshape(T, C, BC).transpose(2, 0, 1))
    return np.ascontiguousarray(np.concatenate(parts, axis=0))


_NC_CACHE = {}


# evac_width=1024 measured ~14% faster in loop probes but crashed the exec
# unit (NRT_EXEC_UNIT_UNRECOVERABLE) on a single-shot run -- two-bank PSUM
# evacuation reads are not safe on this runtime. Stay at 512.
BEST_CONFIG = dict(evac="mixed", evac_width=512)


def kernel(x, w0, b0, w1, b1, w2, b2):
    from concourse.bass_utils import run_bass_kernel_spmd

    if "nc" not in _NC_CACHE:
        _NC_CACHE["nc"] = build_program(**BEST_CONFIG)
    nc = _NC_CACHE["nc"]
    in_maps = prep_inputs(x, w0, b0, w1, b1, w2, b2)
    res = run_bass_kernel_spmd(nc, in_maps, core_ids=list(range(N_CORES)))
    return gather_output(res.results)


# revision 18
# speedup vs baseline: 15.2043x; 1.9787x over previous
"""Trainium2 Bass kernel for per-node masked MLP (gnn_message_passing).

Reference computation (B=8192 batch, T=128 nodes, H=64 hidden, C=2 out):
    h   = leaky_relu(einsum('tij,jt,bj->bti', w0, adj, x) + b0)   adj = 1-eye
    h   = leaky_relu(einsum('tij,btj->bti', w1, h) + b1)
    out = einsum('tij,btj->bti', w2, h) + b2

Strategy: data-parallel over batch across 8 NeuronCores (1024 rows each).
Per core, all three layers are TensorE matmuls with the (t,i) axes on PSUM
partitions and batch streaming on the moving free dim (fp32r -> full-rate
PE). All weights are preloaded into SBUF once (12 MB resident).
  L0: [j=128, ti-tile=128] stationary per 128-wide ti block (self-loop mask
      folded into the weights host-side).
  L1: block-diagonal [W1[2m].T (+) W1[2m+1].T] stationary per node pair.
  L2: 128-wide stationary accumulating 32 node pairs into one PSUM bank
      (each pair owns a distinct 4-column strip).
Bias + leaky-relu ride the PSUM->SBUF evacuation, balanced across ScalarE
(Lrelu) and VectorE (fused custom DVE op, or a 2-op fallback).
"""

import sys

if "/opt/trn_rl_repo" not in sys.path:
    sys.path.insert(0, "/opt/trn_rl_repo")

import numpy as np

B = 8192
T = 128
H = 64
C = 2
N_CORES = 8
BC = B // N_CORES  # 1024 batch rows per core
M_TILES = 64  # 128-wide (t,i) tiles for L0 == node pairs for L1/L2
NEG = 0.01  # leaky_relu negative slope

LEAKY_OP_NAME = "LEAKY_BIAS_ANT"


def _register_leaky_op():
    """Custom fused DVE op: out = max(in0 + s0, in0*imm2 + s1).
    With s0=bias, s1=NEG*bias, imm2=NEG this is leaky_relu(in0+bias) in one
    VectorE pass. Registered into concourse.dve_ops.OPS at runtime."""
    import concourse.dve_ops as dve_ops
    from concourse.dve_ops import DveOp
    from concourse.dve_spec import C0, C1, C2, Spec, Src0, lower, maxx
    from concourse.dve_uop import DveOpSpec

    for op in dve_ops.OPS:
        if op.name == LEAKY_OP_NAME:
            return op

    def _reference(in0, in1, s0, s1, imm2):
        z = in0.astype(np.float32)
        return np.maximum(z + s0, z * imm2 + s1).astype(np.float32)

    spec = Spec(body=maxx(Src0 + C0, Src0 * C2 + C1), reference=_reference)
    row = dve_ops._CUSTOM_DVE_ROW_BASE + len(dve_ops.OPS)
    assert row < 0x20
    shas = {}
    for ver in ("v3", "v4"):
        s = DveOpSpec(
            name=LEAKY_OP_NAME, opcode=row, uops=lower(spec, ver=ver), rd1_en=False
        )
        shas[ver] = s.sha(ver)
    op = DveOp(LEAKY_OP_NAME, spec, subdim=False, uops_sha=shas)
    dve_ops.OPS.append(op)
    dve_ops._SUB_OPCODE_FOR_NAME[LEAKY_OP_NAME] = row
    dve_ops.CUSTOM_DVE_SPECS[LEAKY_OP_NAME] = spec
    return op


def _split_sync_waits(nc, cap=1):
    """This container's walrus build encodes at most ~1 sync wait per
    instruction (setupSyncWait: "Too many sync wait commands"), while Tile's
    sem assignment freely attaches several. Post-pass: leave `cap` waits on
    each instruction and hoist the extras onto single-wait NOPs inserted
    just before it on the same engine (same-engine FIFO preserves
    semantics)."""
    from concourse import mybir

    ctr = [0]
    for f in nc.m.functions:
        for blk in f.blocks:
            new_list = []
            for ins in blk.instructions:
                si = getattr(ins, "sync_info", None)
                waits = list(si.on_wait) if si is not None and si.on_wait else []
                if len(waits) > cap:
                    keep = waits[:cap]
                    extra = waits[cap:]
                    for w in extra:
                        ctr[0] += 1
                        nop = mybir.InstNoOp(
                            name=f"{ins.name}-ws{ctr[0]}",
                            engine=ins.engine,
                            ins=[],
                            outs=[],
                            sync_info=mybir.SyncInfo(on_wait=[w], on_update=[]),
                        )
                        new_list.append(nop)
                    ins.sync_info = mybir.SyncInfo(
                        on_wait=keep, on_update=list(si.on_update or [])
                    )
                new_list.append(ins)
            blk.instructions[:] = new_list


def build_program(loop_R=None, evac="mixed", l2="wide", dve_frac=None, evac_width=512, skew=False, wait_cap=1):
    """Build the per-core Bass program.

    loop_R: wrap the body in a hardware For_i loop (wall-clock slope timing).
    evac:  "mixed" (ACT Lrelu + custom DVE op, balanced), "act", "dvec"
           (custom DVE only), "dve" (2-op DVE only), "none" (timing probe:
           matmuls run on stale h tiles, no evacuation work).
    l2:    "wide" | "off" (timing probe).
    """
    import concourse.bass as bass
    import concourse.tile as tile
    from concourse import mybir

    f32 = mybir.dt.float32
    f32r = mybir.dt.float32r
    Alu = mybir.AluOpType
    Act = mybir.ActivationFunctionType

    nc = bass.Bass()
    xt_d = nc.dram_tensor("xt", [T, BC], f32r, kind="ExternalInput")
    w0_d = nc.dram_tensor("w0w", [T, M_TILES * T], f32r, kind="ExternalInput")
    w1_d = nc.dram_tensor("w1w", [T, M_TILES * T], f32r, kind="ExternalInput")
    w2_d = nc.dram_tensor("w2w", [T, M_TILES * T], f32r, kind="ExternalInput")
    b0_d = nc.dram_tensor("b0s", [T, M_TILES], f32, kind="ExternalInput")
    b1_d = nc.dram_tensor("b1s", [T, M_TILES], f32, kind="ExternalInput")
    b1p_d = nc.dram_tensor("b1p", [T, M_TILES], f32, kind="ExternalInput")
    b0p_d = nc.dram_tensor("b0p", [T, M_TILES], f32, kind="ExternalInput")
    b2_d = nc.dram_tensor("b2s", [T, 2], f32, kind="ExternalInput")
    out_d = nc.dram_tensor("out", [2 * T, BC], f32, kind="ExternalOutput")

    with tile.TileContext(nc) as tc:
        with (
            tc.tile_pool(name="const", bufs=1) as cp,
            tc.tile_pool(name="h0p", bufs=2) as h0p,
            tc.tile_pool(name="h1p", bufs=2) as h1p,
            tc.tile_pool(name="tmp", bufs=3) as tmpp,
            tc.tile_pool(name="outp", bufs=2) as outp,
            tc.tile_pool(
                name="psA", bufs=(4 if skew else 3), space=bass.MemorySpace.PSUM
            ) as psA,
            tc.tile_pool(
                name="psB", bufs=(2 if skew else 3), space=bass.MemorySpace.PSUM
            ) as psB,
            tc.tile_pool(name="psCa", bufs=1, space=bass.MemorySpace.PSUM) as psCa,
            tc.tile_pool(name="psCb", bufs=1, space=bass.MemorySpace.PSUM) as psCb,
        ):
            # ---- resident tensors (loaded once) ----
            xtt = cp.tile([T, BC], f32r, tag="xt")
            nc.sync.dma_start(xtt[:], xt_d[:])
            w0sb = cp.tile([T, M_TILES * T], f32r, tag="w0w")
            nc.sync.dma_start(w0sb[:], w0_d[:])
            w1sb = cp.tile([T, M_TILES * T], f32r, tag="w1w")
            nc.sync.dma_start(w1sb[:], w1_d[:])
            w2sb = cp.tile([T, M_TILES * T], f32r, tag="w2w")
            nc.sync.dma_start(w2sb[:], w2_d[:])
            b0t = cp.tile([T, M_TILES], f32, tag="b0")
            nc.sync.dma_start(b0t[:], b0_d[:])
            b0pt = cp.tile([T, M_TILES], f32, tag="b0p")
            nc.sync.dma_start(b0pt[:], b0p_d[:])
            b1t = cp.tile([T, M_TILES], f32, tag="b1")
            nc.sync.dma_start(b1t[:], b1_d[:])
            b1pt = cp.tile([T, M_TILES], f32, tag="b1p")
            nc.sync.dma_start(b1pt[:], b1p_d[:])
            b2t = cp.tile([T, 2], f32, tag="b2")
            nc.sync.dma_start(b2t[:], b2_d[:])

            if evac == "none":
                h0fix = cp.tile([T, BC], f32r, tag="h0fix")
                nc.gpsimd.memset(h0fix[:].bitcast(f32), 0.125)
                h1fix = cp.tile([T, BC], f32r, tag="h1fix")
                nc.gpsimd.memset(h1fix[:].bitcast(f32), 0.125)
            if evac == "empty":
                scratch = cp.tile([T, 16], f32, tag="scratch")

            # ACT Lrelu ~570ns vs DVE 2-op ~1316ns per [128,512] tile:
            # give VectorE ~31% of the evacuations (4/13).
            # (A fused custom DVE op would halve the DVE cost, but this
            # walrus build rejects InstCustomDveAnt: "ISA wrong length".)
            if dve_frac is None:
                n_dve, n_mod = (4, 13) if evac_width == 512 else (3, 10)
            else:
                n_dve, n_mod = dve_frac
            ctr = [0]

            def evac_leaky(dst, ps, bias_col, bias01_col):
                k = ctr[0]
                ctr[0] += 1
                use_dve = (evac in ("dvec", "dve")) or (
                    evac == "mixed" and k % n_mod < n_dve
                )
                if use_dve:
                    t1 = tmpp.tile([T, evac_width], f32, tag="t1")
                    nc.vector.tensor_scalar(
                        t1[:], ps, NEG, bias01_col, op0=Alu.mult, op1=Alu.add
                    )
                    nc.vector.scalar_tensor_tensor(
                        dst, ps, bias_col, t1[:], op0=Alu.add, op1=Alu.max
                    )
                else:
                    nc.scalar.activation(
                        dst, ps, Act.Lrelu, bias=bias_col, scale=1.0, alpha=NEG
                    )

            state = {}

            def stage_l0(m):
                w0t = w0sb[:, T * m : T * (m + 1)]
                if evac_width == 1024:
                    ps0 = psA.tile([T, 1024], f32, tag="ps")
                    nc.tensor.matmul(
                        ps0[:, 0:512], w0t, xtt[:, 0:512], start=True, stop=True
                    )
                    nc.tensor.matmul(
                        ps0[:, 512:1024], w0t, xtt[:, 512:1024],
                        start=True, stop=True,
                    )
                    ps0_parts = [(ps0[:, 0:1024], slice(0, 1024))]
                else:
                    ps0a = psA.tile([T, 512], f32, tag="ps0")
                    nc.tensor.matmul(
                        ps0a[:], w0t, xtt[:, 0:512], start=True, stop=True
                    )
                    ps0b = psA.tile([T, 512], f32, tag="ps0")
                    nc.tensor.matmul(
                        ps0b[:], w0t, xtt[:, 512:1024], start=True, stop=True
                    )
                    ps0_parts = [(ps0a[:], slice(0, 512)), (ps0b[:], slice(512, 1024))]
                if evac == "none":
                    h0 = h0fix
                else:
                    h0 = h0p.tile([T, BC], f32r, tag="h0")
                    for ps_ap, sl in ps0_parts:
                        evac_leaky(h0[:, sl], ps_ap, b0t[:, m : m + 1], b0pt[:, m : m + 1])
                state[("h0", m)] = h0

            def stage_l12(m):
                g, mq = divmod(m, 32)
                w1t = w1sb[:, T * m : T * (m + 1)]
                w2t = w2sb[:, T * m : T * (m + 1)]
                h0 = state.pop(("h0", m))
                if l2 == "wide" and mq == 0:
                    ps2a = psCa.tile([T, 512], f32, tag="ps2a")
                    ps2b = psCb.tile([T, 512], f32, tag="ps2b")
                    state["ps2"] = (ps2a, ps2b)
                if evac_width == 1024:
                    ps1 = psA.tile([T, 1024], f32, tag="ps")
                    nc.tensor.matmul(
                        ps1[:, 0:512], w1t, h0[:, 0:512], start=True, stop=True
                    )
                    nc.tensor.matmul(
                        ps1[:, 512:1024], w1t, h0[:, 512:1024],
                        start=True, stop=True,
                    )
                    ps1_parts = [(ps1[:, 0:1024], slice(0, 1024))]
                else:
                    ps1a = psB.tile([T, 512], f32, tag="ps1")
                    nc.tensor.matmul(
                        ps1a[:], w1t, h0[:, 0:512], start=True, stop=True
                    )
                    ps1b = psB.tile([T, 512], f32, tag="ps1")
                    nc.tensor.matmul(
                        ps1b[:], w1t, h0[:, 512:1024], start=True, stop=True
                    )
                    ps1_parts = [(ps1a[:], slice(0, 512)), (ps1b[:], slice(512, 1024))]
                if evac == "none":
                    h1 = h1fix
                else:
                    h1 = h1p.tile([T, BC], f32r, tag="h1")
                    for ps_ap, sl in ps1_parts:
                        evac_leaky(h1[:, sl], ps_ap, b1t[:, m : m + 1], b1pt[:, m : m + 1])
                if l2 == "wide":
                    ps2a, ps2b = state["ps2"]
                    nc.tensor.matmul(
                        ps2a[:], w2t, h1[:, 0:512],
                        start=(mq == 0), stop=(mq == 31),
                    )
                    nc.tensor.matmul(
                        ps2b[:], w2t, h1[:, 512:1024],
                        start=(mq == 0), stop=(mq == 31),
                    )
                    if mq == 31:
                        oa = outp.tile([T, 512], f32, tag="oa")
                        nc.scalar.activation(
                            oa[:], ps2a[:], Act.Identity, bias=b2t[:, g : g + 1]
                        )
                        nc.sync.dma_start(
                            out_d[128 * g : 128 * (g + 1), 0:512], oa[:]
                        )
                        ob = outp.tile([T, 512], f32, tag="ob")
                        nc.scalar.activation(
                            ob[:], ps2b[:], Act.Identity, bias=b2t[:, g : g + 1]
                        )
                        nc.sync.dma_start(
                            out_d[128 * g : 128 * (g + 1), 512:1024], ob[:]
                        )

            def body(_iv=None):
                if evac == "empty":
                    nc.gpsimd.memset(scratch[:], 0.0)
                    return
                if skew:
                    for m in range(M_TILES + 1):
                        if m < M_TILES:
                            stage_l0(m)
                        if m >= 1:
                            stage_l12(m - 1)
                else:
                    for m in range(M_TILES):
                        stage_l0(m)
                        stage_l12(m)

            if loop_R is None:
                body()
            else:
                with tc.For_i(0, loop_R, 1) as iv:
                    body(iv)

            if evac == "empty" or l2 == "off":
                # timing probes never write out_d in the body; bind it so the
                # output tensor isn't dead
                z = cp.tile([T, 16], f32, tag="zpad")
                nc.gpsimd.memset(z[:], 0.0)
                nc.sync.dma_start(out_d[0:T, 0:16], z[:])

    _split_sync_waits(nc, cap=wait_cap)
    return nc


def prep_inputs(x, w0, b0, w1, b1, w2, b2):
    """Host-side reshuffle of the full inputs into the per-core tensors."""
    x = np.ascontiguousarray(np.asarray(x, dtype=np.float32))
    w0 = np.asarray(w0, dtype=np.float32)
    b0 = np.asarray(b0, dtype=np.float32)
    w1 = np.asarray(w1, dtype=np.float32)
    b1 = np.asarray(b1, dtype=np.float32)
    w2 = np.asarray(w2, dtype=np.float32)
    b2 = np.asarray(b2, dtype=np.float32)

    # L0 stationaries: mask self-loop; [j, (m p)] with column 128m+p -> ti
    w0m = w0.copy()
    w0m[np.arange(T), :, np.arange(T)] = 0.0
    w0w = np.ascontiguousarray(w0m.transpose(2, 0, 1).reshape(T, T * H))

    # L1 stationaries: block-diag of the pair's transposed weights
    w1T = w1.transpose(0, 2, 1)  # [t, i_in, i_out]
    w1s = np.zeros((M_TILES, T, T), np.float32)
    w1s[:, :H, :H] = w1T[0::2]
    w1s[:, H:, H:] = w1T[1::2]
    w1w = np.ascontiguousarray(w1s.transpose(1, 0, 2).reshape(T, M_TILES * T))

    # L2 stationaries: pair m owns columns 4*(m%32) .. +4
    w2T = w2.transpose(0, 2, 1)  # [t, i, c]
    w2s = np.zeros((M_TILES, T, T), np.float32)
    for m in range(M_TILES):
        col = 4 * (m % 32)
        w2s[m, :H, col : col + C] = w2T[2 * m]
        w2s[m, H:, col + C : col + 2 * C] = w2T[2 * m + 1]
    w2w = np.ascontiguousarray(w2s.transpose(1, 0, 2).reshape(T, M_TILES * T))

    b0s = np.ascontiguousarray(b0.reshape(-1).reshape(M_TILES, T).T)
    b1s = np.ascontiguousarray(b1.reshape(-1).reshape(M_TILES, T).T)
    b2s = np.ascontiguousarray(b2.reshape(-1).reshape(2, T).T)

    shared = {
        "w0w": w0w, "w1w": w1w, "w2w": w2w,
        "b0s": b0s, "b0p": np.ascontiguousarray(NEG * b0s),
        "b1s": b1s, "b1p": np.ascontiguousarray(NEG * b1s),
        "b2s": b2s,
    }
    in_maps = []
    for c in range(N_CORES):
        xt_c = np.ascontiguousarray(x[c * BC : (c + 1) * BC].T)  # [128, BC]
        in_maps.append({"xt": xt_c, **shared})
    return in_maps


def gather_output(results):
    """results: list of per-core {"out": [256, BC]} -> full [B, T, C]."""
    parts = []
    for c in range(N_CORES):
        o = np.asarray(results[c]["out"])  # [2T, BC], row r = t*2+c
        parts.append(o.reshape(T, C, BC).transpose(2, 0, 1))
    return np.ascontiguousarray(np.concatenate(parts, axis=0))


_NC_CACHE = {}


# evac_width=1024 measured ~14% faster in loop probes but crashed the exec
# unit (NRT_EXEC_UNIT_UNRECOVERABLE) on a single-shot run -- two-bank PSUM
# evacuation reads are not safe on this runtime. Stay at 512.
# skew=True (software pipeline: L0 of iter m issued before L1/L2 of m-1)
# measured 433 us/iter vs 482 us/iter unskewed, rel err 2.2e-4 on HW.
BEST_CONFIG = dict(evac="mixed", evac_width=512, skew=True)


def kernel(x, w0, b0, w1, b1, w2, b2):
    from concourse.bass_utils import run_bass_kernel_spmd

    if "nc" not in _NC_CACHE:
        _NC_CACHE["nc"] = build_program(**BEST_CONFIG)
    nc = _NC_CACHE["nc"]
    in_maps = prep_inputs(x, w0, b0, w1, b1, w2, b2)
    res = run_bass_kernel_spmd(nc, in_maps, core_ids=list(range(N_CORES)))
    return gather_output(res.results)
